# revision 31
# baseline (speedup 1.0000x reference)
"""Trainium2 Bass kernel for nn_Body2_interaction (SO(3) 2-body self-interaction).

Computation per node n (N=50000 nodes, 9 spherical components m, 128 channels):
  xl = so3_linear(x, W_left, b_left)     # per-degree block-diag channel mixing
  xr = so3_linear(x, W_right, b_right)
  tp = channelwise CG tensor product (uuu) of xl, xr over instrs
       [(2,2,0), (1,2,1), (1,1,2)]
  out = so3_linear(tp, W_final, b_final)

Strategy (pure node-parallel over 8 cores, per sharding hint):
  - transpose x tiles to [channel, node] layout on the PE (channels must be on
    partitions for the channel-contraction matmuls)
  - left/right linears as matmuls with pre-transposed weights
  - CG tensor product: 25 distinct elementwise products on DVE; the
    (w3j * alpha)-weighted accumulation over products is FOLDED into the final
    linear: 27 PSUM-accumulated matmuls with pre-scaled copies of W_final^T
  - transpose the result back to node-major and DMA out
"""

import numpy as np
from math import factorial
from contextlib import ExitStack

import concourse.bass as bass
import concourse.bacc as bacc
import concourse.tile as tile
from concourse import mybir
from concourse.ap import AP as _AP
from concourse.bass_utils import run_bass_kernel_spmd

# ----------------------------------------------------------------------------
# problem constants (hardcoded per spec)
N_FULL = 50000
N_CORES = 8
M_DIM = 9
C = 128
INSTR = [(2, 2, 0), (1, 2, 1), (1, 1, 2)]
L_OF_M = [0, 1, 1, 1, 2, 2, 2, 2, 2]  # degree of each global m row

N_PER_CORE = 6400           # padded: 8 * 6400 = 51200 >= 50000
S = 256                     # supertile (nodes); 6400 = 25 * 256
CHUNK = 128                 # transpose chunk (partition dim)
N_SUPER = N_PER_CORE // S
N_CHUNKS = S // CHUNK

F32 = mybir.dt.float32
F32R = mybir.dt.float32r
BF16 = mybir.dt.bfloat16

# ---- dtype knobs ------------------------------------------------------------
LIN_DT = F32R          # dtype of XT + left/right weights (linear matmuls)
PROD_DT = BF16         # dtype of xl/xr/products/wf (product + fused-final path)

# ----------------------------------------------------------------------------
# Wigner 3j (real basis, e3nn convention) -- from the reference definition


def _f(n):
    return float(factorial(int(n)))


def _su2_cg(j1, m1, j2, m2, j3, m3):
    if m3 != m1 + m2:
        return 0.0
    vmin = int(max(-j1 + j2 + m3, -j1 + m1, 0))
    vmax = int(min(j2 + j3 + m1, j3 - j1 + j2, j3 + m3))
    pref = ((2 * j3 + 1) * _f(j3 + j1 - j2) * _f(j3 - j1 + j2) * _f(j1 + j2 - j3) / _f(j1 + j2 + j3 + 1)
            * _f(j3 + m3) * _f(j3 - m3) / (_f(j1 - m1) * _f(j1 + m1) * _f(j2 - m2) * _f(j2 + m2))) ** 0.5
    s = 0.0
    for v in range(vmin, vmax + 1):
        s += (-1.0) ** (v + j2 + m2) / _f(v) * _f(j2 + j3 + m1 - v) * _f(j1 - m1 + v) / (
            _f(j3 - j1 + j2 - v) * _f(j3 + m3 - v) * _f(v + j1 - j2 - m3))
    return pref * s


def _q_real_to_complex(l):
    q = np.zeros((2 * l + 1, 2 * l + 1), dtype=np.complex128)
    for m in range(-l, 0):
        q[l + m, l + abs(m)] = 1.0 / np.sqrt(2)
        q[l + m, l - abs(m)] = -1j / np.sqrt(2)
    q[l, l] = 1.0
    for m in range(1, l + 1):
        q[l + m, l + abs(m)] = (-1) ** m / np.sqrt(2)
        q[l + m, l - abs(m)] = 1j * (-1) ** m / np.sqrt(2)
    return (-1j) ** l * q


def wigner_3j(l1, l2, l3):
    Csu2 = np.zeros((2 * l1 + 1, 2 * l2 + 1, 2 * l3 + 1), dtype=np.complex128)
    for i, m1 in enumerate(range(-l1, l1 + 1)):
        for k, m2 in enumerate(range(-l2, l2 + 1)):
            for n, m3 in enumerate(range(-l3, l3 + 1)):
                Csu2[i, k, n] = _su2_cg(l1, m1, l2, m2, l3, m3)
    Q1, Q2, Q3 = _q_real_to_complex(l1), _q_real_to_complex(l2), _q_real_to_complex(l3)
    Cx = np.einsum('ij,kl,mn,ikn->jlm', Q1, Q2, np.conj(Q3.T), Csu2)
    Cx = Cx.real
    return Cx / np.linalg.norm(Cx)


def build_tp_tables():
    """Returns (products, terms_by_k):
    products: list of distinct (gi, gj) global-row product pairs
    terms_by_k: dict k -> list of (product_index, coef) with coef = alpha*w3j
    """
    prod_index = {}
    products = []
    terms_by_k = {k: [] for k in range(M_DIM)}
    for (l1, l2, lo) in INSTR:
        w3j = wigner_3j(l1, l2, lo)
        alpha = float((2 * lo + 1) ** 0.5)
        base_i, base_j, base_k = l1 * l1, l2 * l2, lo * lo
        nz = np.argwhere(np.abs(w3j) > 1e-10)
        for (i, j, k) in nz:
            gi, gj, gk = base_i + int(i), base_j + int(j), base_k + int(k)
            coef = alpha * float(w3j[i, j, k])
            if (gi, gj) not in prod_index:
                prod_index[(gi, gj)] = len(products)
                products.append((gi, gj))
            terms_by_k[gk].append((prod_index[(gi, gj)], coef))
    return products, terms_by_k


PRODUCTS, TERMS_BY_K = build_tp_tables()
NP_PLANES = len(PRODUCTS)  # 25
ALL_TERMS = [(k, p, coef) for k in range(M_DIM) for (p, coef) in TERMS_BY_K[k]]
NT = len(ALL_TERMS)  # 27

# ----------------------------------------------------------------------------
# device program


def build_program(n_nodes):
    assert n_nodes % S == 0
    nc = bacc.Bacc("TRN2", target_bir_lowering=False)

    x_d = nc.dram_tensor("x", [n_nodes, M_DIM * C], LIN_DT, kind="ExternalInput")
    wl_d = nc.dram_tensor("wlt", [C, 3 * C], LIN_DT, kind="ExternalInput")
    wr_d = nc.dram_tensor("wrt", [C, 3 * C], LIN_DT, kind="ExternalInput")
    wf_d = nc.dram_tensor("wft", [C, NT * C], PROD_DT, kind="ExternalInput")
    id_d = nc.dram_tensor("ident", [C, C], F32, kind="ExternalInput")
    idr_d = nc.dram_tensor("identr", [C, C], LIN_DT, kind="ExternalInput")
    bl_d = nc.dram_tensor("bl", [C, 1], F32, kind="ExternalInput")
    br_d = nc.dram_tensor("br", [C, 1], F32, kind="ExternalInput")
    bf_d = nc.dram_tensor("bf", [C, 1], F32, kind="ExternalInput")
    out_d = nc.dram_tensor("out", [n_nodes, M_DIM * C], F32, kind="ExternalOutput")

    Ident = mybir.ActivationFunctionType.Identity

    with tile.TileContext(nc) as tc, ExitStack() as ctx:
        consts = ctx.enter_context(tc.tile_pool(name="consts", bufs=1))
        xpool = ctx.enter_context(tc.tile_pool(name="xpool", bufs=3))
        mids = ctx.enter_context(tc.tile_pool(name="mids", bufs=2))
        opool = ctx.enter_context(tc.tile_pool(name="opool", bufs=2))
        ps_in_pool = ctx.enter_context(tc.tile_pool(name="psin", bufs=2, space="PSUM"))
        ps_lin_pool = ctx.enter_context(tc.tile_pool(name="pslin", bufs=2, space="PSUM"))
        ps_fin_pool = ctx.enter_context(tc.tile_pool(name="psfin", bufs=2, space="PSUM"))
        ps_out_pool = ctx.enter_context(tc.tile_pool(name="psout", bufs=2, space="PSUM"))

        wl_s = consts.tile([C, 3 * C], LIN_DT)
        nc.sync.dma_start(wl_s[:], wl_d[:])
        wr_s = consts.tile([C, 3 * C], LIN_DT)
        nc.sync.dma_start(wr_s[:], wr_d[:])
        wf_s = consts.tile([C, NT * C], PROD_DT)
        nc.sync.dma_start(wf_s[:], wf_d[:])
        id_s = consts.tile([C, C], F32)
        nc.sync.dma_start(id_s[:], id_d[:])
        id_r = consts.tile([C, C], LIN_DT)
        nc.sync.dma_start(id_r[:], idr_d[:])
        bl_s = consts.tile([C, 1], F32)
        nc.sync.dma_start(bl_s[:], bl_d[:])
        br_s = consts.tile([C, 1], F32)
        nc.sync.dma_start(br_s[:], br_d[:])
        bf_s = consts.tile([C, 1], F32)
        nc.sync.dma_start(bf_s[:], bf_d[:])

        # --- warmup: make PE/ACT observe the const DMA semaphores early ---
        ps_w = ps_out_pool.tile([C, 4 * C], F32, tag="psout")
        nc.tensor.transpose(ps_w[:, 0:C], id_s[:], id_s[:])
        nc.tensor.transpose(ps_w[:, C:2 * C].bitcast(LIN_DT), id_r[:], id_r[:])
        nc.tensor.transpose(ps_w[:, 2 * C:3 * C].bitcast(LIN_DT), wl_s[:, 0:C], id_r[:])
        nc.tensor.transpose(ps_w[:, 3 * C:4 * C].bitcast(LIN_DT), wr_s[:, 0:C], id_r[:])
        warm_s = consts.tile([C, 4], F32)
        nc.scalar.copy(warm_s[:], _AP(ps_w.tensor, ps_w.offset,
                                      [list(ps_w.ap[0]), [C, 4]]))
        warm_b = consts.tile([C, 3], F32)
        nc.scalar.copy(warm_b[:, 0:1], bl_s[:])
        nc.scalar.copy(warm_b[:, 1:2], br_s[:])
        nc.vector.tensor_copy(warm_b[:, 2:3], bf_s[:])

        # --- greedy ACT/DVE load balancer for PSUM->SBUF copies ---
        eng_ns = {"act": 0.0, "dve": 0.0}

        def copy_psum(dst, src, fd, bias=None):
            a = eng_ns["act"] + (260 + fd) / 1.2
            d = eng_ns["dve"] + (120 + fd) / 0.96
            if a <= d:
                eng_ns["act"] = a
                if bias is not None:
                    nc.scalar.activation(dst, src, Ident, bias=bias, scale=1.0)
                else:
                    nc.scalar.copy(dst, src)
            else:
                eng_ns["dve"] = d
                if bias is not None:
                    nc.vector.tensor_scalar_add(dst, src, bias)
                else:
                    nc.vector.tensor_copy(dst, src)

        def note_dve(fd):
            eng_ns["dve"] += (58 + fd / 2) / 0.96

        # m-plane pairs for batched psum tiles
        M_PAIRS = [(0, 1), (2, 3), (4, 5), (6, 7), (8,)]
        K_BATCHES = ((0, 1, 2, 3), (4, 5, 6, 7), (8,))
        T_BASE = [sum(len(TERMS_BY_K[kk]) for kk in range(k)) for k in range(M_DIM)]
        WMAX = 2 * S

        def process(gi_idx, base, W):
            n_half = W // S

            # ---- load + in-transpose, per 256-node half ----
            XT = mids.tile([C, M_DIM * WMAX], LIN_DT, tag="XT", bufs=1,
                           name=f"XT_{gi_idx}")
            for h in range(n_half):
                hb = base + h * S
                xnat = xpool.tile([CHUNK, N_CHUNKS * M_DIM * C], LIN_DT, tag="xnat",
                                  name=f"xnat_{gi_idx}_{h}")
                x_view = x_d[hb: hb + S, :].rearrange("(t p) c -> p t c", p=CHUNK)
                nc.sync.dma_start(xnat[:], x_view)
                for mp in M_PAIRS:
                    ps_in = ps_in_pool.tile([C, 2 * S], LIN_DT, tag="psin",
                                            name=f"psin_{gi_idx}_{h}_{mp[0]}")
                    for j, m in enumerate(mp):
                        for t in range(N_CHUNKS):
                            nc.tensor.transpose(
                                ps_in[:, j * S + t * CHUNK: j * S + (t + 1) * CHUNK],
                                xnat[:, (t * M_DIM + m) * C: (t * M_DIM + m + 1) * C],
                                id_r[:],
                            )
                    # scatter the 1-2 m-planes into W-wide m-major layout
                    dst = _AP(XT.tensor,
                              XT.offset + mp[0] * W + h * S,
                              [list(XT.ap[0]), [W, len(mp)], [1, S]])
                    copy_psum(dst, ps_in[:, 0: len(mp) * S], len(mp) * S)

            # ---- left / right linears over full W-wide planes ----
            XL = mids.tile([C, M_DIM * WMAX], PROD_DT, tag="XL", bufs=1,
                           name=f"XL_{gi_idx}")
            XR = mids.tile([C, M_DIM * WMAX], PROD_DT, tag="XR", bufs=1,
                           name=f"XR_{gi_idx}")
            for (w_s, b_s, XX) in ((wl_s, bl_s, XL), (wr_s, br_s, XR)):
                for m in range(M_DIM):
                    ps_l = ps_lin_pool.tile([C, WMAX], F32, tag="pslin",
                                            name=f"pslin_{gi_idx}_{m}")
                    nc.tensor.matmul(
                        ps_l[:, 0:W],
                        w_s[:, L_OF_M[m] * C:(L_OF_M[m] + 1) * C],
                        XT[:, m * W: m * W + W],
                        start=True, stop=True,
                    )
                    copy_psum(XX[:, m * W: m * W + W], ps_l[:, 0:W], W,
                              bias=(b_s[:] if m == 0 else None))

            # ---- channelwise products (bf16, DVE) ----
            P = mids.tile([C, NP_PLANES * WMAX], PROD_DT, tag="P",
                          name=f"P_{gi_idx}")
            for p, (gp, gq) in enumerate(PRODUCTS):
                nc.vector.tensor_mul(
                    P[:, p * WMAX: p * WMAX + W],
                    XL[:, gp * W: gp * W + W],
                    XR[:, gq * W: gq * W + W],
                )
                note_dve(W)

            # ---- fused TP-accumulate + final linear ----
            FOUT = mids.tile([C, M_DIM * WMAX], F32, tag="FOUT",
                             name=f"FOUT_{gi_idx}")
            for k in range(M_DIM):
                terms = TERMS_BY_K[k]
                ps_f = ps_fin_pool.tile([C, WMAX], F32, tag="psfin",
                                        name=f"psfin_{gi_idx}_{k}")
                for i, (p, _coef) in enumerate(terms):
                    nc.tensor.matmul(
                        ps_f[:, 0:W],
                        wf_s[:, (T_BASE[k] + i) * C:(T_BASE[k] + i + 1) * C],
                        P[:, p * WMAX: p * WMAX + W],
                        start=(i == 0), stop=(i == len(terms) - 1),
                    )
                copy_psum(FOUT[:, k * WMAX: k * WMAX + W], ps_f[:, 0:W], W,
                          bias=(bf_s[:] if k == 0 else None))

            # ---- transpose back to node-major; one DMA per 256-node half ----
            for h in range(n_half):
                hb = base + h * S
                onat = opool.tile([CHUNK, N_CHUNKS * M_DIM * C], F32, tag="onat",
                                  name=f"onat_{gi_idx}_{h}")
                for t in range(N_CHUNKS):
                    ct = h * S + t * CHUNK
                    for kb in K_BATCHES:
                        ps_o = ps_out_pool.tile([CHUNK, 4 * C], F32, tag="psout",
                                                name=f"psout_{gi_idx}_{h}_{t}_{kb[0]}")
                        for ki, k in enumerate(kb):
                            nc.tensor.transpose(
                                ps_o[:, ki * C:(ki + 1) * C],
                                FOUT[:, k * WMAX + ct: k * WMAX + ct + CHUNK],
                                id_s[:],
                            )
                        copy_psum(
                            onat[:, (t * M_DIM + kb[0]) * C:(t * M_DIM + kb[-1] + 1) * C],
                            ps_o[:, 0:len(kb) * C], len(kb) * C)
                o_view = out_d[hb: hb + S, :].rearrange("(t p) c -> p t c", p=CHUNK)
                nc.sync.dma_start(o_view, onat[:])

        n_pairs, tail = divmod(n_nodes, WMAX)
        for g in range(n_pairs):
            process(g, g * WMAX, WMAX)
        if tail:
            assert tail == S
            process(n_pairs, n_pairs * WMAX, S)

    nc.compile()
    return nc


# ----------------------------------------------------------------------------
# host wrapper

_CACHED = {}


def _get_program(n_nodes):
    if n_nodes not in _CACHED:
        _CACHED[n_nodes] = build_program(n_nodes)
    return _CACHED[n_nodes]


def _host_inputs(W_left, b_left, W_right, b_right, W_final, b_final):
    wlt = np.ascontiguousarray(W_left.transpose(2, 0, 1).reshape(C, 3 * C)).astype(np.float32)
    wrt = np.ascontiguousarray(W_right.transpose(2, 0, 1).reshape(C, 3 * C)).astype(np.float32)
    # fused final weights: per term (k, p, coef): coef * W_final[lo(k)]^T  -> [c, t, d]
    wft = np.zeros((C, NT, C), dtype=np.float32)
    for ti, (k, p, coef) in enumerate(ALL_TERMS):
        lo = L_OF_M[k]
        wft[:, ti, :] = coef * np.asarray(W_final[lo], dtype=np.float32).T
    wft = np.ascontiguousarray(wft.reshape(C, NT * C))
    if PROD_DT == BF16:
        import ml_dtypes
        wft = wft.astype(ml_dtypes.bfloat16)
    return {
        "wlt": wlt,
        "wrt": wrt,
        "wft": wft,
        "ident": np.eye(C, dtype=np.float32),
        "identr": np.eye(C, dtype=np.float32),
        "bl": np.asarray(b_left, np.float32).reshape(C, 1),
        "br": np.asarray(b_right, np.float32).reshape(C, 1),
        "bf": np.asarray(b_final, np.float32).reshape(C, 1),
    }


def kernel(irreps_x, W_left, b_left, W_right, b_right, W_final, b_final):
    irreps_x = np.asarray(irreps_x, dtype=np.float32)
    n = irreps_x.shape[0]
    n_pad = N_CORES * N_PER_CORE
    x = np.zeros((n_pad, M_DIM * C), dtype=np.float32)
    x[:n] = irreps_x.reshape(n, M_DIM * C)

    weights = _host_inputs(W_left, b_left, W_right, b_right, W_final, b_final)
    in_maps = []
    for i in range(N_CORES):
        m = dict(weights)
        m["x"] = np.ascontiguousarray(x[i * N_PER_CORE:(i + 1) * N_PER_CORE])
        in_maps.append(m)

    nc = _get_program(N_PER_CORE)
    res = run_bass_kernel_spmd(nc, in_maps, core_ids=list(range(N_CORES)))
    global LAST_RESULTS
    LAST_RESULTS = res
    out = np.concatenate([r["out"] for r in res.results], axis=0)
    return out[:n].reshape(n, M_DIM, C)


LAST_RESULTS = None


# revision 41
# speedup vs baseline: 1.1668x; 1.1668x over previous
"""Trainium2 Bass kernel for nn_Body2_interaction (SO(3) 2-body self-interaction).

Computation per node n (N=50000 nodes, 9 spherical components m, 128 channels):
  xl = so3_linear(x, W_left, b_left)     # per-degree block-diag channel mixing
  xr = so3_linear(x, W_right, b_right)
  tp = channelwise CG tensor product (uuu) of xl, xr over instrs
       [(2,2,0), (1,2,1), (1,1,2)]
  out = so3_linear(tp, W_final, b_final)

Strategy (pure node-parallel over 8 cores, per sharding hint):
  - transpose x tiles to [channel, node] layout on the PE (channels must be on
    partitions for the channel-contraction matmuls)
  - left/right linears as matmuls with pre-transposed weights
  - CG tensor product: 25 distinct elementwise products on DVE; the
    (w3j * alpha)-weighted accumulation over products is FOLDED into the final
    linear: 27 PSUM-accumulated matmuls with pre-scaled copies of W_final^T
  - transpose the result back to node-major and DMA out
"""

import numpy as np
from math import factorial
from contextlib import ExitStack

import concourse.bass as bass
import concourse.bacc as bacc
import concourse.tile as tile
from concourse import mybir
from concourse.ap import AP as _AP
from concourse.bass_utils import run_bass_kernel_spmd

# ----------------------------------------------------------------------------
# problem constants (hardcoded per spec)
N_FULL = 50000
N_CORES = 8
M_DIM = 9
C = 128
INSTR = [(2, 2, 0), (1, 2, 1), (1, 1, 2)]
L_OF_M = [0, 1, 1, 1, 2, 2, 2, 2, 2]  # degree of each global m row

N_PER_CORE = 6400           # padded: 8 * 6400 = 51200 >= 50000
S = 256                     # supertile (nodes); 6400 = 25 * 256
CHUNK = 128                 # transpose chunk (partition dim)
N_SUPER = N_PER_CORE // S
N_CHUNKS = S // CHUNK

F32 = mybir.dt.float32
F32R = mybir.dt.float32r
BF16 = mybir.dt.bfloat16

# ---- dtype knobs ------------------------------------------------------------
LIN_DT = BF16          # dtype of x/XT + left/right weights (input + linears)
PROD_DT = BF16         # dtype of xl/xr/products/wf (product + fused-final path)

# ----------------------------------------------------------------------------
# Wigner 3j (real basis, e3nn convention) -- from the reference definition


def _f(n):
    return float(factorial(int(n)))


def _su2_cg(j1, m1, j2, m2, j3, m3):
    if m3 != m1 + m2:
        return 0.0
    vmin = int(max(-j1 + j2 + m3, -j1 + m1, 0))
    vmax = int(min(j2 + j3 + m1, j3 - j1 + j2, j3 + m3))
    pref = ((2 * j3 + 1) * _f(j3 + j1 - j2) * _f(j3 - j1 + j2) * _f(j1 + j2 - j3) / _f(j1 + j2 + j3 + 1)
            * _f(j3 + m3) * _f(j3 - m3) / (_f(j1 - m1) * _f(j1 + m1) * _f(j2 - m2) * _f(j2 + m2))) ** 0.5
    s = 0.0
    for v in range(vmin, vmax + 1):
        s += (-1.0) ** (v + j2 + m2) / _f(v) * _f(j2 + j3 + m1 - v) * _f(j1 - m1 + v) / (
            _f(j3 - j1 + j2 - v) * _f(j3 + m3 - v) * _f(v + j1 - j2 - m3))
    return pref * s


def _q_real_to_complex(l):
    q = np.zeros((2 * l + 1, 2 * l + 1), dtype=np.complex128)
    for m in range(-l, 0):
        q[l + m, l + abs(m)] = 1.0 / np.sqrt(2)
        q[l + m, l - abs(m)] = -1j / np.sqrt(2)
    q[l, l] = 1.0
    for m in range(1, l + 1):
        q[l + m, l + abs(m)] = (-1) ** m / np.sqrt(2)
        q[l + m, l - abs(m)] = 1j * (-1) ** m / np.sqrt(2)
    return (-1j) ** l * q


def wigner_3j(l1, l2, l3):
    Csu2 = np.zeros((2 * l1 + 1, 2 * l2 + 1, 2 * l3 + 1), dtype=np.complex128)
    for i, m1 in enumerate(range(-l1, l1 + 1)):
        for k, m2 in enumerate(range(-l2, l2 + 1)):
            for n, m3 in enumerate(range(-l3, l3 + 1)):
                Csu2[i, k, n] = _su2_cg(l1, m1, l2, m2, l3, m3)
    Q1, Q2, Q3 = _q_real_to_complex(l1), _q_real_to_complex(l2), _q_real_to_complex(l3)
    Cx = np.einsum('ij,kl,mn,ikn->jlm', Q1, Q2, np.conj(Q3.T), Csu2)
    Cx = Cx.real
    return Cx / np.linalg.norm(Cx)


def build_tp_tables():
    """Returns (products, terms_by_k):
    products: list of distinct (gi, gj) global-row product pairs
    terms_by_k: dict k -> list of (product_index, coef) with coef = alpha*w3j
    """
    prod_index = {}
    products = []
    terms_by_k = {k: [] for k in range(M_DIM)}
    for (l1, l2, lo) in INSTR:
        w3j = wigner_3j(l1, l2, lo)
        alpha = float((2 * lo + 1) ** 0.5)
        base_i, base_j, base_k = l1 * l1, l2 * l2, lo * lo
        nz = np.argwhere(np.abs(w3j) > 1e-10)
        for (i, j, k) in nz:
            gi, gj, gk = base_i + int(i), base_j + int(j), base_k + int(k)
            coef = alpha * float(w3j[i, j, k])
            if (gi, gj) not in prod_index:
                prod_index[(gi, gj)] = len(products)
                products.append((gi, gj))
            terms_by_k[gk].append((prod_index[(gi, gj)], coef))
    return products, terms_by_k


PRODUCTS, TERMS_BY_K = build_tp_tables()
NP_PLANES = len(PRODUCTS)  # 25
ALL_TERMS = [(k, p, coef) for k in range(M_DIM) for (p, coef) in TERMS_BY_K[k]]
NT = len(ALL_TERMS)  # 27

# ----------------------------------------------------------------------------
# device program


def build_program(n_nodes):
    assert n_nodes % S == 0
    nc = bacc.Bacc("TRN2", target_bir_lowering=False)

    x_d = nc.dram_tensor("x", [n_nodes, M_DIM * C], LIN_DT, kind="ExternalInput")
    wl_d = nc.dram_tensor("wlt", [C, 3 * C], LIN_DT, kind="ExternalInput")
    wr_d = nc.dram_tensor("wrt", [C, 3 * C], LIN_DT, kind="ExternalInput")
    wf_d = nc.dram_tensor("wft", [C, NT * C], PROD_DT, kind="ExternalInput")
    id_d = nc.dram_tensor("ident", [C, C], F32, kind="ExternalInput")
    idr_d = nc.dram_tensor("identr", [C, C], LIN_DT, kind="ExternalInput")
    bl_d = nc.dram_tensor("bl", [C, 1], F32, kind="ExternalInput")
    br_d = nc.dram_tensor("br", [C, 1], F32, kind="ExternalInput")
    bf_d = nc.dram_tensor("bf", [C, 1], F32, kind="ExternalInput")
    out_d = nc.dram_tensor("out", [n_nodes, M_DIM * C], F32, kind="ExternalOutput")

    Ident = mybir.ActivationFunctionType.Identity

    with tile.TileContext(nc) as tc, ExitStack() as ctx:
        consts = ctx.enter_context(tc.tile_pool(name="consts", bufs=1))
        xpool = ctx.enter_context(tc.tile_pool(name="xpool", bufs=4))
        mids = ctx.enter_context(tc.tile_pool(name="mids", bufs=2))
        opool = ctx.enter_context(tc.tile_pool(name="opool", bufs=3))
        ps_in_pool = ctx.enter_context(tc.tile_pool(name="psin", bufs=2, space="PSUM"))
        ps_lin_pool = ctx.enter_context(tc.tile_pool(name="pslin", bufs=2, space="PSUM"))
        ps_fin_pool = ctx.enter_context(tc.tile_pool(name="psfin", bufs=2, space="PSUM"))
        ps_out_pool = ctx.enter_context(tc.tile_pool(name="psout", bufs=2, space="PSUM"))

        wl_s = consts.tile([C, 3 * C], LIN_DT)
        nc.sync.dma_start(wl_s[:], wl_d[:])
        wr_s = consts.tile([C, 3 * C], LIN_DT)
        nc.sync.dma_start(wr_s[:], wr_d[:])
        wf_s = consts.tile([C, NT * C], PROD_DT)
        nc.sync.dma_start(wf_s[:], wf_d[:])
        id_s = consts.tile([C, C], F32)
        nc.sync.dma_start(id_s[:], id_d[:])
        id_r = consts.tile([C, C], LIN_DT)
        nc.sync.dma_start(id_r[:], idr_d[:])
        bl_s = consts.tile([C, 1], F32)
        nc.sync.dma_start(bl_s[:], bl_d[:])
        br_s = consts.tile([C, 1], F32)
        nc.sync.dma_start(br_s[:], br_d[:])
        bf_s = consts.tile([C, 1], F32)
        nc.sync.dma_start(bf_s[:], bf_d[:])

        # --- warmup: make PE/ACT observe the const DMA semaphores early ---
        ps_w = ps_out_pool.tile([C, 4 * C], F32, tag="psout")
        nc.tensor.transpose(ps_w[:, 0:C], id_s[:], id_s[:])
        nc.tensor.transpose(ps_w[:, C:C + C // 2].bitcast(LIN_DT), id_r[:], id_r[:])
        nc.tensor.transpose(ps_w[:, 2 * C:2 * C + C // 2].bitcast(LIN_DT), wl_s[:, 0:C], id_r[:])
        nc.tensor.transpose(ps_w[:, 3 * C:3 * C + C // 2].bitcast(LIN_DT), wr_s[:, 0:C], id_r[:])
        warm_s = consts.tile([C, 4], F32)
        nc.scalar.copy(warm_s[:], _AP(ps_w.tensor, ps_w.offset,
                                      [list(ps_w.ap[0]), [C, 4]]))
        warm_b = consts.tile([C, 3], F32)
        nc.scalar.copy(warm_b[:, 0:1], bl_s[:])
        nc.scalar.copy(warm_b[:, 1:2], br_s[:])
        nc.vector.tensor_copy(warm_b[:, 2:3], bf_s[:])

        # --- greedy ACT/DVE load balancer for PSUM->SBUF copies ---
        eng_ns = {"act": 0.0, "dve": 0.0}

        def copy_psum(dst, src, fd, bias=None):
            a = eng_ns["act"] + (260 + fd) / 1.2
            d = eng_ns["dve"] + (120 + fd) / 0.96
            if a <= d:
                eng_ns["act"] = a
                if bias is not None:
                    nc.scalar.activation(dst, src, Ident, bias=bias, scale=1.0)
                else:
                    nc.scalar.copy(dst, src)
            else:
                eng_ns["dve"] = d
                if bias is not None:
                    nc.vector.tensor_scalar_add(dst, src, bias)
                else:
                    nc.vector.tensor_copy(dst, src)

        def note_dve(fd):
            eng_ns["dve"] += (58 + fd / 2) / 0.96

        # m-plane pairs for batched psum tiles
        M_PAIRS = [(0, 1), (2, 3), (4, 5), (6, 7), (8,)]
        K_BATCHES = ((0, 1, 2, 3), (4, 5, 6, 7), (8,))
        T_BASE = [sum(len(TERMS_BY_K[kk]) for kk in range(k)) for k in range(M_DIM)]

        for st in range(n_nodes // S):
            base = st * S

            # ---- load both 128-node chunks in one DMA; partition = node ----
            xnat = xpool.tile([CHUNK, N_CHUNKS * M_DIM * C], LIN_DT, tag="xnat",
                              name=f"xnat_{st}")
            x_view = x_d[base: base + S, :].rearrange("(t p) c -> p t c", p=CHUNK)
            nc.sync.dma_start(xnat[:], x_view)

            # ---- transpose to XT[c, m*S + node] (m-major planes) ----
            XT = mids.tile([C, M_DIM * S], LIN_DT, tag="XT", name=f"XT_{st}")
            for mp in M_PAIRS:
                ps_in = ps_in_pool.tile([C, 2 * S], LIN_DT, tag="psin",
                                        name=f"psin_{st}_{mp[0]}")
                for j, m in enumerate(mp):
                    for t in range(N_CHUNKS):
                        nc.tensor.transpose(
                            ps_in[:, j * S + t * CHUNK: j * S + (t + 1) * CHUNK],
                            xnat[:, (t * M_DIM + m) * C: (t * M_DIM + m + 1) * C],
                            id_r[:],
                        )
                copy_psum(XT[:, mp[0] * S: (mp[-1] + 1) * S],
                          ps_in[:, 0: len(mp) * S], len(mp) * S)

            # ---- left / right linears (bf16 out for the product stage) ----
            XL = mids.tile([C, M_DIM * S], PROD_DT, tag="XL", name=f"XL_{st}")
            XR = mids.tile([C, M_DIM * S], PROD_DT, tag="XR", name=f"XR_{st}")
            for (w_s, b_s, XX) in ((wl_s, bl_s, XL), (wr_s, br_s, XR)):
                for mp in M_PAIRS:
                    ps_l = ps_lin_pool.tile([C, 2 * S], F32, tag="pslin",
                                            name=f"pslin_{st}_{mp[0]}")
                    for j, m in enumerate(mp):
                        nc.tensor.matmul(
                            ps_l[:, j * S:(j + 1) * S],
                            w_s[:, L_OF_M[m] * C:(L_OF_M[m] + 1) * C],
                            XT[:, m * S:(m + 1) * S],
                            start=True, stop=True,
                        )
                    if mp[0] == 0:
                        # bias applies to the m=0 plane only
                        copy_psum(XX[:, 0:S], ps_l[:, 0:S], S, bias=b_s[:])
                        copy_psum(XX[:, S:2 * S], ps_l[:, S:2 * S], S)
                    else:
                        copy_psum(XX[:, mp[0] * S:(mp[-1] + 1) * S],
                                  ps_l[:, 0:len(mp) * S], len(mp) * S)

            # ---- channelwise products (bf16, DVE) ----
            P = mids.tile([C, NP_PLANES * S], PROD_DT, tag="P", name=f"P_{st}")
            for p, (gi, gj) in enumerate(PRODUCTS):
                nc.vector.tensor_mul(
                    P[:, p * S:(p + 1) * S],
                    XL[:, gi * S:(gi + 1) * S],
                    XR[:, gj * S:(gj + 1) * S],
                )
                note_dve(S)

            # ---- fused TP-accumulate + final linear ----
            FOUT = mids.tile([C, M_DIM * S], F32, tag="FOUT", name=f"FOUT_{st}")
            for kp in M_PAIRS:
                ps_f = ps_fin_pool.tile([C, 2 * S], F32, tag="psfin",
                                        name=f"psfin_{st}_{kp[0]}")
                for j, k in enumerate(kp):
                    terms = TERMS_BY_K[k]
                    for i, (p, _coef) in enumerate(terms):
                        nc.tensor.matmul(
                            ps_f[:, j * S:(j + 1) * S],
                            wf_s[:, (T_BASE[k] + i) * C:(T_BASE[k] + i + 1) * C],
                            P[:, p * S:(p + 1) * S],
                            start=(i == 0), stop=(i == len(terms) - 1),
                        )
                if kp[0] == 0:
                    copy_psum(FOUT[:, 0:S], ps_f[:, 0:S], S, bias=bf_s[:])
                    copy_psum(FOUT[:, S:2 * S], ps_f[:, S:2 * S], S)
                else:
                    copy_psum(FOUT[:, kp[0] * S:(kp[-1] + 1) * S],
                              ps_f[:, 0:len(kp) * S], len(kp) * S)

            # ---- transpose back to node-major; single output DMA ----
            onat = opool.tile([CHUNK, N_CHUNKS * M_DIM * C], F32, tag="onat",
                              name=f"onat_{st}")
            for t in range(N_CHUNKS):
                for kb in K_BATCHES:
                    ps_o = ps_out_pool.tile([CHUNK, 4 * C], F32, tag="psout",
                                            name=f"psout_{st}_{t}_{kb[0]}")
                    for ki, k in enumerate(kb):
                        nc.tensor.transpose(
                            ps_o[:, ki * C:(ki + 1) * C],
                            FOUT[:, k * S + t * CHUNK: k * S + (t + 1) * CHUNK],
                            id_s[:],
                        )
                    copy_psum(
                        onat[:, (t * M_DIM + kb[0]) * C:(t * M_DIM + kb[-1] + 1) * C],
                        ps_o[:, 0:len(kb) * C], len(kb) * C)
            o_view = out_d[base: base + S, :].rearrange("(t p) c -> p t c", p=CHUNK)
            nc.sync.dma_start(o_view, onat[:])

    nc.compile()
    return nc


# ----------------------------------------------------------------------------
# host wrapper

_CACHED = {}


def _get_program(n_nodes):
    if n_nodes not in _CACHED:
        _CACHED[n_nodes] = build_program(n_nodes)
    return _CACHED[n_nodes]


def _np_dt(dt):
    if dt == BF16:
        import ml_dtypes
        return ml_dtypes.bfloat16
    return np.float32


def _host_inputs(W_left, b_left, W_right, b_right, W_final, b_final):
    lin_np = _np_dt(LIN_DT)
    wlt = np.ascontiguousarray(W_left.transpose(2, 0, 1).reshape(C, 3 * C)).astype(lin_np)
    wrt = np.ascontiguousarray(W_right.transpose(2, 0, 1).reshape(C, 3 * C)).astype(lin_np)
    # fused final weights: per term (k, p, coef): coef * W_final[lo(k)]^T  -> [c, t, d]
    wft = np.zeros((C, NT, C), dtype=np.float32)
    for ti, (k, p, coef) in enumerate(ALL_TERMS):
        lo = L_OF_M[k]
        wft[:, ti, :] = coef * np.asarray(W_final[lo], dtype=np.float32).T
    wft = np.ascontiguousarray(wft.reshape(C, NT * C))
    if PROD_DT == BF16:
        import ml_dtypes
        wft = wft.astype(ml_dtypes.bfloat16)
    return {
        "wlt": wlt,
        "wrt": wrt,
        "wft": wft,
        "ident": np.eye(C, dtype=np.float32),
        "identr": np.eye(C, dtype=_np_dt(LIN_DT)),
        "bl": np.asarray(b_left, np.float32).reshape(C, 1),
        "br": np.asarray(b_right, np.float32).reshape(C, 1),
        "bf": np.asarray(b_final, np.float32).reshape(C, 1),
    }


def kernel(irreps_x, W_left, b_left, W_right, b_right, W_final, b_final):
    irreps_x = np.asarray(irreps_x, dtype=np.float32)
    n = irreps_x.shape[0]
    n_pad = N_CORES * N_PER_CORE
    x = np.zeros((n_pad, M_DIM * C), dtype=_np_dt(LIN_DT))
    x[:n] = irreps_x.reshape(n, M_DIM * C).astype(_np_dt(LIN_DT))

    weights = _host_inputs(W_left, b_left, W_right, b_right, W_final, b_final)
    in_maps = []
    for i in range(N_CORES):
        m = dict(weights)
        m["x"] = np.ascontiguousarray(x[i * N_PER_CORE:(i + 1) * N_PER_CORE])
        in_maps.append(m)

    nc = _get_program(N_PER_CORE)
    res = run_bass_kernel_spmd(nc, in_maps, core_ids=list(range(N_CORES)))
    global LAST_RESULTS
    LAST_RESULTS = res
    out = np.concatenate([r["out"] for r in res.results], axis=0)
    return out[:n].reshape(n, M_DIM, C)


LAST_RESULTS = None
